# revision 17
# baseline (speedup 1.0000x reference)
"""Trainium2 Bass kernel for a cross-attention layer (CoAttention + RMSNorm output).

Reference computation (per batch b):
    q = hidden @ Wq.T + bq ; k = kv @ Wk.T + bk ; v = kv @ Wv.T + bv
    probs = softmax(q k^T / sqrt(64))
    ctx = probs @ v
    out = RMSNorm(ctx @ Wo.T + bo + hidden) * gamma

Sharding: 8 cores = 4 batches x 2 query-row halves. Each core produces
out[b, half*1024:(half+1)*1024, :] with no cross-core communication
(K/V projections are duplicated within a batch pair).

Per-core pipeline (all matmuls keep contraction dim on SBUF partitions,
enabled by host-side transposes of activations/weights):
  A) QT[o, s]  = WqT.T @ xqT          (fp32r, +bq via per-partition bias)
  B) KT[o, s]  = WkT.T @ xkvT -> DRAM scratch (streamed back per head pair)
  C) V[s, o]   = xkvT.T @ WvT -> SBUF resident as bf16 [kv, head, 64+1]
                 (65th column = 1.0: folds softmax row-sums into ctx matmul)
  D) per head: S^T[kv, sq] = KT_h.T @ QT_h (fp32r; two heads packed in the
     128-row PE array via base-partition 0/64), E = exp(S^T/8) in bf16 on ACT,
     ctx+^T[65, sq] += V+_h.T @ E (bf16); row 64 accumulates sum(exp).
     Normalize: R = broadcast(1/rowsum) via a K=1 PE matmul; ctxT = ctx+ * R.
  E) O[s, o] = ctxT.T @ WoT (bf16) + resid(+bo); RMSNorm * gamma; DMA out.
"""

import numpy as np
import ml_dtypes

import concourse.bass as bass
import concourse.mybir as mybir
from concourse import bass_utils, tile

P = 128
H = 1024
NH = 16
HD = 64
B = 4
SQ = 2048
SQL = 1024  # per-core query rows
SKV = 2048
KC = H // P  # 8 contraction chunks of 128
NKV = SKV // P  # 16 kv chunks
EPS = 1e-6

F32 = mybir.dt.float32
F32R = mybir.dt.float32r
BF16 = mybir.dt.bfloat16
AF = mybir.ActivationFunctionType
OP = mybir.AluOpType

N_CORES = 8


class SplitDrainTileContext(tile.TileContext):
    """TileContext whose tail drain splits sem waits across chained drains.

    The walrus build in this container rejects CTRL instructions that carry
    more than one sync wait; the stock tail drain aggregates the whole global
    clock onto a single Drain instruction.
    """

    MAXW = 1

    def _drain_and_barrier(self, tick_clock, wait_clock):
        drain_inst = self.nc.sync.drain()
        wait_clock.add_sem_waits(
            drain_inst.ins, tile.ScopedClock({None: tick_clock.global_clock})
        )
        si = drain_inst.ins.sync_info
        if si is not None and si.on_wait and len(si.on_wait) > self.MAXW:
            waits = list(si.on_wait)
            drain_inst.ins.sync_info = mybir.SyncInfo(
                on_wait=waits[: self.MAXW], on_update=list(si.on_update or [])
            )
            rest = waits[self.MAXW :]
            for i in range(0, len(rest), self.MAXW):
                d2 = self.nc.sync.drain()
                d2.ins.sync_info = mybir.SyncInfo(
                    on_wait=rest[i : i + self.MAXW], on_update=[]
                )
        self.nc.all_engine_barrier()
        assert self.sems is not None
        popped = self.nc._tile_sem_poison_stack.pop()
        assert popped is self._sem_poison
        self.nc.clear_and_free_semaphores(list(self.sems.allocated().values()))
        self.nc.all_engine_barrier()


def _split_sync_waits(nc, maxw=1):
    """Hoist excess per-instruction sem waits onto preceding same-engine NoOps.

    The walrus build in this container rejects instructions carrying more
    than one sync wait command (any opcode family)."""
    n = 0
    tail_bb = nc.cur_bb.bb
    for f in nc.m.functions:
        for bb in f.blocks:
            il = bb.instructions
            i = 0
            while i < len(il):
                inst = il[i]
                si = inst.sync_info
                if si is not None and si.on_wait and len(si.on_wait) > maxw:
                    waits = list(si.on_wait)
                    keep = waits[-maxw:]
                    extra = waits[:-maxw]
                    inst.sync_info = mybir.SyncInfo(
                        on_wait=keep, on_update=list(si.on_update or [])
                    )
                    for w in extra:
                        b = nc.engines[inst.engine].nop(nofuse=True)
                        carrier = b.ins
                        popped = tail_bb.instructions.pop()
                        assert popped is carrier, "nop landed in unexpected block"
                        carrier.sync_info = mybir.SyncInfo(on_wait=[w], on_update=[])
                        il.insert(i, carrier)
                        i += 1
                        n += 1
                i += 1
    return n


def build_core_kernel(split_waits=True):
    nc = bass.Bass("TRN2", target_bir_lowering=False, debug=False, num_devices=1)

    def inp(name, shape, dt=F32):
        return nc.dram_tensor(name, shape, dt, kind="ExternalInput").ap()

    xqT = inp("xqT", [H, SQL], BF16)
    xkvT = inp("xkvT", [H, SKV], BF16)
    wqT = inp("wqT", [H, H], BF16)
    wkT = inp("wkT", [H, H], BF16)
    wvT = inp("wvT", [H, H], BF16)
    woT = inp("woT", [H, H], BF16)
    bqc = inp("bqc", [P, KC])
    bkc = inp("bkc", [P, KC])
    bvr = inp("bvr", [P, H])
    resid = inp("resid", [SQL, H])
    gam = inp("gam", [P, H])
    onesd = inp("onesd", [1, HD], F32R)
    out = nc.dram_tensor("out", [SQL, H], F32, kind="ExternalOutput").ap()

    with SplitDrainTileContext(nc) as tc:
        with (
            nc.allow_low_precision(reason="fp32r staging of fp32 data"),
            tc.tile_pool(name="dram", bufs=1, space="DRAM") as dpool,
            tc.tile_pool(name="pers", bufs=1) as pers,
            tc.tile_pool(name="wt", bufs=1) as wpool,
            tc.tile_pool(name="xs", bufs=2) as xpool,
            tc.tile_pool(name="kts", bufs=2) as ktpool,
            tc.tile_pool(name="sm", bufs=4) as smpool,
            tc.tile_pool(name="r4", bufs=3) as rpool,
            tc.tile_pool(name="tiny", bufs=4) as tpool,
            tc.tile_pool(name="ps", bufs=4, space="PSUM") as pspool,
            tc.tile_pool(name="ps2", bufs=2, space="PSUM") as ps2pool,
        ):
            kt_d = dpool.tile([H, SKV], BF16, name="ktd")

            # --- persistent tiles -------------------------------------------------
            qt = pers.tile([P, KC, SQL], BF16, name="qt")          # Q^T  [o, s]
            v_sb = pers.tile([P, NKV, NH, HD + 1], BF16, name="v_sb")
            ctxT = pers.tile([P, KC, SQL], BF16, name="ctxT")     # ctx^T [c, s]
            bq_sb = pers.tile([P, KC], F32, name="bq_sb")
            bk_sb = pers.tile([P, KC], F32, name="bk_sb")
            bv_sb = pers.tile([P, H], F32, name="bv_sb")
            gam_sb = pers.tile([P, H], F32, name="gam_sb")
            ones1 = pers.tile([1, HD], F32R, name="ones1")
            eps_sb = pers.tile([P, 1], F32, name="eps_sb")
            nc.vector.memset(eps_sb, EPS)

            nc.sync.dma_start(bq_sb, bqc)
            nc.sync.dma_start(bk_sb, bkc)
            nc.sync.dma_start(bv_sb, bvr)
            nc.sync.dma_start(gam_sb, gam)
            nc.sync.dma_start(ones1, onesd)
            nc.vector.memset(v_sb[:, :, :, HD], 1.0)

            # --- phase A: Q^T = WqT.T @ xqT (+bq) ---------------------------------
            wq = wpool.tile([P, KC, H], BF16, tag="wt", name="wq")
            for ic in range(KC):
                nc.sync.dma_start(wq[:, ic, :], wqT[ic * P : (ic + 1) * P, :])
            for sc in range(SQL // 512):
                xq = xpool.tile([P, KC, 512], BF16, tag="xs", name="xq")
                for ic in range(KC):
                    nc.sync.dma_start(
                        xq[:, ic, :], xqT[ic * P : (ic + 1) * P, sc * 512 : (sc + 1) * 512]
                    )
                for oc in range(KC):
                    ps = pspool.tile([P, 512], F32, tag="p512", name="ps_q")
                    for ic in range(KC):
                        nc.tensor.matmul(
                            ps,
                            wq[:, ic, oc * P : (oc + 1) * P],
                            xq[:, ic, :],
                            start=(ic == 0),
                            stop=(ic == KC - 1),
                        )
                    nc.vector.tensor_scalar_add(
                        qt[:, oc, sc * 512 : (sc + 1) * 512], ps, bq_sb[:, oc : oc + 1]
                    )

            # --- phase B: K^T = WkT.T @ xkvT (+bk) -> DRAM ------------------------
            wk = wpool.tile([P, KC, H], BF16, tag="wt", name="wk")
            for ic in range(KC):
                nc.sync.dma_start(wk[:, ic, :], wkT[ic * P : (ic + 1) * P, :])
            for sc in range(SKV // 512):
                xkv = xpool.tile([P, KC, 512], BF16, tag="xs", name="xkv")
                for ic in range(KC):
                    nc.sync.dma_start(
                        xkv[:, ic, :], xkvT[ic * P : (ic + 1) * P, sc * 512 : (sc + 1) * 512]
                    )
                for oc in range(KC):
                    ps = pspool.tile([P, 512], F32, tag="p512", name="ps_k")
                    for ic in range(KC):
                        nc.tensor.matmul(
                            ps,
                            wk[:, ic, oc * P : (oc + 1) * P],
                            xkv[:, ic, :],
                            start=(ic == 0),
                            stop=(ic == KC - 1),
                        )
                    st = smpool.tile([P, 512], BF16, tag="sm", name="kst")
                    nc.vector.tensor_scalar_add(st, ps, bk_sb[:, oc : oc + 1])
                    nc.sync.dma_start(
                        kt_d[oc * P : (oc + 1) * P, sc * 512 : (sc + 1) * 512], st
                    )

            # --- phase C: V = xkvT.T @ WvT (+bv) -> bf16 SBUF ---------------------
            wv = wpool.tile([P, KC, H], BF16, tag="wt", name="wv")
            for ic in range(KC):
                nc.sync.dma_start(wv[:, ic, :], wvT[ic * P : (ic + 1) * P, :])
            for sc in range(SKV // 512):
                xkv2 = xpool.tile([P, KC, 512], BF16, tag="xs", name="xkv2")
                for ic in range(KC):
                    nc.sync.dma_start(
                        xkv2[:, ic, :],
                        xkvT[ic * P : (ic + 1) * P, sc * 512 : (sc + 1) * 512],
                    )
                for s2 in range(4):
                    kvc = sc * 4 + s2
                    for oc2 in range(2):
                        ps = pspool.tile([P, 512], F32, tag="p512", name="ps_v")
                        for ic in range(KC):
                            nc.tensor.matmul(
                                ps,
                                xkv2[:, ic, s2 * P : (s2 + 1) * P],
                                wv[:, ic, oc2 * 512 : (oc2 + 1) * 512],
                                start=(ic == 0),
                                stop=(ic == KC - 1),
                            )
                        nc.vector.tensor_tensor(
                            v_sb[:, kvc, oc2 * 8 : (oc2 + 1) * 8, 0:HD],
                            ps.rearrange("p (h d) -> p h d", d=HD),
                            bvr_view(bv_sb, oc2),
                            OP.add,
                        )

            # --- phase D: attention per head pair ---------------------------------
            for hp in range(NH // 2):
                ktp = ktpool.tile([P, SKV], BF16, tag="kts", name="ktp")
                nc.sync.dma_start(ktp, kt_d[hp * P : (hp + 1) * P, :])
                cps = {}
                for h in range(2):
                    for sqc in range(2):
                        cps[(h, sqc)] = pspool.tile(
                            [P, 512], F32, tag="p512", name="ps_ctx"
                        )
                for kvc in range(NKV):
                    for h in range(2):
                        sp = ps2pool.tile([P, 1024], F32, tag="p1024", name="ps_sc")
                        for sqc in range(2):
                            nc.tensor.matmul(
                                sp[:, sqc * 512 : (sqc + 1) * 512],
                                ktp[h * HD : (h + 1) * HD, kvc * P : (kvc + 1) * P],
                                
                                    qt[
                                        h * HD : (h + 1) * HD,
                                        hp,
                                        sqc * 512 : (sqc + 1) * 512,
                                    ]
                                ,
                                start=True,
                                stop=True,
                            )
                        e = smpool.tile([P, 1024], BF16, tag="sm", name="e_t")
                        nc.scalar.activation(e, sp, AF.Exp, scale=0.125)
                        for sqc in range(2):
                            nc.tensor.matmul(
                                cps[(h, sqc)][0 : HD + 1, :],
                                v_sb[:, kvc, 2 * hp + h, :],
                                e[:, sqc * 512 : (sqc + 1) * 512],
                                start=(kvc == 0),
                                stop=(kvc == NKV - 1),
                            )
                # normalize: ctxT = ctx+ * broadcast(1/rowsum)
                for h in range(2):
                    rp = ps2pool.tile([P, 1024], F32, tag="p1024", name="ps_r")
                    for sqc in range(2):
                        rec = smpool.tile([1, 512], F32R, tag="sm", name="rec")
                        nc.vector.reciprocal(rec, cps[(h, sqc)][HD : HD + 1, :])
                        nc.tensor.matmul(
                            rp[0:HD, sqc * 512 : (sqc + 1) * 512],
                            ones1,
                            rec,
                            start=True,
                            stop=True,
                        )
                    r_sb = xpool.tile([HD, SQL], F32, tag="xs", name="r_sb")
                    nc.vector.tensor_copy(r_sb, rp[0:HD, :])
                    for sqc in range(2):
                        dst = ctxT[
                            h * HD : (h + 1) * HD, hp, sqc * 512 : (sqc + 1) * 512
                        ]
                        src0 = cps[(h, sqc)][0:HD, :]
                        src1 = r_sb[:, sqc * 512 : (sqc + 1) * 512]
                        if h == 0:
                            nc.vector.tensor_tensor(dst, src0, src1, OP.mult)
                        else:
                            stg = smpool.tile([HD, 512], BF16, tag="sm", name="stg")
                            nc.vector.tensor_tensor(stg, src0, src1, OP.mult)
                            nc.sync.dma_start(dst, stg)

            # --- phase E: O-proj + residual + RMSNorm -----------------------------
            wo = wpool.tile([P, KC, H], BF16, tag="wt", name="wo")
            for ic in range(KC):
                nc.sync.dma_start(wo[:, ic, :], woT[ic * P : (ic + 1) * P, :])
            for s2 in range(SQL // P):
                rs = rpool.tile([P, H], F32, tag="r4", name="rs")
                nc.sync.dma_start(rs, resid[s2 * P : (s2 + 1) * P, :])
                h_sb = rpool.tile([P, H], F32, tag="r4", name="h_sb")
                for oc2 in range(2):
                    pso = pspool.tile([P, 512], F32, tag="p512", name="ps_o")
                    for cc in range(KC):
                        nc.tensor.matmul(
                            pso,
                            ctxT[:, cc, s2 * P : (s2 + 1) * P],
                            wo[:, cc, oc2 * 512 : (oc2 + 1) * 512],
                            start=(cc == 0),
                            stop=(cc == KC - 1),
                        )
                    nc.vector.tensor_tensor(
                        h_sb[:, oc2 * 512 : (oc2 + 1) * 512],
                        pso,
                        rs[:, oc2 * 512 : (oc2 + 1) * 512],
                        OP.add,
                    )
                sq = xpool.tile([P, H], F32, tag="xs", name="sq_scratch")
                ss = tpool.tile([P, 1], F32, tag="tiny", name="ss")
                nc.vector.tensor_tensor(sq, h_sb, h_sb, OP.mult)
                nc.vector.tensor_reduce(ss, sq, axis=mybir.AxisListType.X, op=OP.add)
                sr = tpool.tile([P, 1], F32, tag="tiny", name="sr")
                nc.scalar.activation(sr, ss, AF.Sqrt, scale=1.0 / H, bias=eps_sb)
                rr = tpool.tile([P, 1], F32, tag="tiny", name="rr")
                nc.vector.reciprocal(rr, sr)
                nc.vector.tensor_scalar_mul(h_sb, h_sb, rr)
                nc.vector.tensor_tensor(rs, h_sb, gam_sb, OP.mult)
                nc.sync.dma_start(out[s2 * P : (s2 + 1) * P, :], rs)

    if split_waits:
        _split_sync_waits(nc)
    return nc


def bvr_view(bv_sb, oc2):
    return bv_sb[:, oc2 * 512 : (oc2 + 1) * 512].rearrange("p (h d) -> p h d", d=HD)


_NC = None


def _get_nc():
    global _NC
    if _NC is None:
        _NC = build_core_kernel()
    return _NC


def make_in_maps(hidden_states, keyvalue_states, Wq, bq, Wk, bk, Wv, bv, Wo, bo, gamma):
    f = np.float32
    hidden_states = np.asarray(hidden_states, f)
    keyvalue_states = np.asarray(keyvalue_states, f)
    shared = {
        "wqT": np.ascontiguousarray(np.asarray(Wq, f).T).astype(ml_dtypes.bfloat16),
        "wkT": np.ascontiguousarray(np.asarray(Wk, f).T).astype(ml_dtypes.bfloat16),
        "wvT": np.ascontiguousarray(np.asarray(Wv, f).T).astype(ml_dtypes.bfloat16),
        "woT": np.ascontiguousarray(np.asarray(Wo, f).T).astype(ml_dtypes.bfloat16),
        "bqc": np.ascontiguousarray(np.asarray(bq, f).reshape(KC, P).T),
        "bkc": np.ascontiguousarray(np.asarray(bk, f).reshape(KC, P).T),
        "bvr": np.ascontiguousarray(np.tile(np.asarray(bv, f), (P, 1))),
        "gam": np.ascontiguousarray(np.tile(np.asarray(gamma, f), (P, 1))),
        "onesd": np.ones((1, HD), f),
    }
    bo = np.asarray(bo, f)
    in_maps = []
    for core in range(N_CORES):
        b, half = divmod(core, 2)
        hq = hidden_states[b, half * SQL : (half + 1) * SQL, :]
        m = dict(shared)
        m["xqT"] = np.ascontiguousarray(hq.T).astype(ml_dtypes.bfloat16)
        m["xkvT"] = np.ascontiguousarray(keyvalue_states[b].T).astype(ml_dtypes.bfloat16)
        m["resid"] = np.ascontiguousarray(hq + bo)
        in_maps.append(m)
    return in_maps


def _run(in_maps, trace=False, **kwargs):
    nc = _get_nc()
    return bass_utils.run_bass_kernel_spmd(
        nc, in_maps, core_ids=list(range(N_CORES)), trace=trace, **kwargs
    )


def _assemble(res):
    out = np.empty((B, SQ, H), np.float32)
    for core in range(N_CORES):
        b, half = divmod(core, 2)
        out[b, half * SQL : (half + 1) * SQL, :] = res.results[core]["out"]
    return out


def kernel(hidden_states, keyvalue_states, Wq, bq, Wk, bk, Wv, bv, Wo, bo, gamma):
    in_maps = make_in_maps(
        hidden_states, keyvalue_states, Wq, bq, Wk, bk, Wv, bv, Wo, bo, gamma
    )
    return _assemble(_run(in_maps))
